# revision 26
# baseline (speedup 1.0000x reference)
"""StyleGAN2-style modulated 3x3 conv (B=8, Ci=Co=512, H=W=32) on 8 TRN2 NeuronCores.

Sharding: data-parallel over batch, one sample per core (embarrassingly
parallel, no collectives). Per core the conv is computed as 9 shifted
matmuls over a zero-padded 34x34 image held in SBUF, contracting over
Ci in 128-chunks with fp32 accumulation in PSUM; compute dtype bf16.

Math (per sample b, with s = (Ci*K*K)**-0.5 folded out of both the conv
and the demod norm so the weights can be used unscaled):
  conv = conv2d(x * y_s, weight)                     # raw, no s
  xs2[o] = sum_i y_s[i]^2 * w2[i,o],  w2 = sum_k weight[o,i,k]^2
  out = conv / sqrt(xs2 + 1e-8 * Ci * K * K) + bias

Host-side prep is layout only plus the input-independent w2 fold
(a pure weight transform, shipped as a 10th "k-slot" of the weight
tensor); all input-dependent math runs on device.
"""

import numpy as np
import ml_dtypes

import concourse.mybir as mybir
from concourse import bacc
from concourse.tile import TileContext
from concourse.bass_utils import run_bass_kernel_spmd

B = 8
CI = 512
CO = 512
H = W = 32
KK = 9  # 3x3
NCI = CI // 128
NCO = CO // 128
HWPAD = 34
EPS_EFF = 1e-8 * CI * KK  # demod eps compensated for unscaled weights

F32 = mybir.dt.float32
BF16 = mybir.dt.bfloat16
AF = mybir.ActivationFunctionType


def build_nc():
    nc = bacc.Bacc("TRN2", target_bir_lowering=False, debug=False)

    x_ext = nc.declare_dram_parameter("x", [NCI, 128, H, W], BF16, isOutput=False)
    # cols 0..3 = y_s per ci-tile, cols 4..7 = bias per co-tile
    yb_ext = nc.declare_dram_parameter("yb", [128, 2 * NCI], F32, isOutput=False)
    # [jci, jco, ci_p, k(9)+w2(1), co_c] bf16
    wt_ext = nc.declare_dram_parameter(
        "wt", [NCI, NCO, 128, KK + 1, 128], BF16, isOutput=False
    )
    out_ext = nc.declare_dram_parameter("out", [NCO, 128, H * W], F32, isOutput=True)

    with TileContext(nc) as tc:
        with (
            tc.tile_pool(name="singles", bufs=1) as singles,
            tc.tile_pool(name="wts", bufs=1) as wts,
            tc.tile_pool(name="pads", bufs=1) as pads,
            tc.tile_pool(name="xin", bufs=4) as xin,
            tc.tile_pool(name="outs", bufs=3) as outs,
            tc.tile_pool(name="cps", bufs=6, space="PSUM") as cps,
            tc.tile_pool(name="dps", bufs=1, space="PSUM") as dps,
            tc.tile_pool(name="wps", bufs=1, space="PSUM") as wps,
        ):
            # ---- input DMAs ----
            # x on SP queues first (needed first); weights issued from the
            # otherwise-idle ACT engine at co-quarter granularity so the
            # first conv groups don't wait on whole-weight transfers.
            xt_sb = []
            yb_sb = singles.tile([128, 2 * NCI], F32)
            for j in range(NCI):
                xt = xin.tile([128, H, W], BF16)
                nc.sync.dma_start(out=xt, in_=x_ext[j])
                xt_sb.append(xt)
                if j == 0:
                    nc.sync.dma_start(out=yb_sb, in_=yb_ext[:, :])
            wt_sb = [[None] * NCO for _ in range(NCI)]
            for q in range(NCO):
                for j in range(NCI):
                    w = wts.tile([128, KK + 1, 128], BF16, tag=f"wt{j}_{q}")
                    nc.scalar.dma_start(out=w, in_=wt_ext[j, q])
                    wt_sb[j][q] = w

            def wt_slice(j, jo, k):
                return wt_sb[j][jo][:, k, :]

            # ---- PE warm-up: ~4us of throwaway matmuls on memset data so the
            # HAM clock gate releases before the real stream starts ----
            warm_lhs = singles.tile([128, 1], BF16)
            nc.vector.memset(warm_lhs, 1.0)
            warm_rhs = singles.tile([128, 512], BF16)
            nc.vector.memset(warm_rhs, 0.5)
            warm_ps = wps.tile([1, 512], F32)
            N_WARM = 8
            for i in range(N_WARM):
                nc.tensor.matmul(
                    out=warm_ps,
                    lhsT=warm_lhs,
                    rhs=warm_rhs,
                    start=(i == 0),
                    stop=(i == N_WARM - 1),
                )

            eps_sb = singles.tile([128, 1], F32)
            nc.vector.memset(eps_sb, EPS_EFF)

            # ---- zero-padded modulated input (bf16), border-only memsets ----
            pad_sb = []
            for j in range(NCI):
                p = pads.tile([128, HWPAD, HWPAD], BF16, tag=f"pad{j}")
                for eng in (nc.gpsimd,):
                    eng.memset(p[:, 0, :], 0.0)
                    eng.memset(p[:, HWPAD - 1, :], 0.0)
                    eng.memset(p[:, 1 : HWPAD - 1, 0], 0.0)
                    eng.memset(p[:, 1 : HWPAD - 1, HWPAD - 1], 0.0)
                pad_sb.append(p)
            for j in range(NCI):
                nc.vector.tensor_scalar(
                    out=pad_sb[j][:, 1 : H + 1, 1 : W + 1],
                    in0=xt_sb[j],
                    scalar1=yb_sb[:, j : j + 1],
                    scalar2=None,
                    op0=mybir.AluOpType.mult,
                )
            # ys^2 in bf16 for the demod matmuls (not needed until ~25us in)
            ys2_sb = singles.tile([128, NCI], BF16)
            nc.vector.tensor_mul(ys2_sb, yb_sb[:, 0:NCI], yb_sb[:, 0:NCI])

            xs2_ps = dps.tile([128, NCO], F32)
            rs_sb = singles.tile([128, NCO], F32)

            def conv_mms(jo, half):
                ps = cps.tile([128, 512], F32, tag="ps")
                h0 = half * 16
                idx = 0
                for j in range(NCI):
                    for k in range(KK):
                        kh, kw = divmod(k, 3)
                        rhs = pad_sb[j][:, kh + h0 : kh + h0 + 16, kw : kw + W]
                        nc.tensor.matmul(
                            out=ps,
                            lhsT=wt_slice(j, jo, k),
                            rhs=rhs,
                            start=(idx == 0),
                            stop=(idx == KK * NCI - 1),
                        )
                        idx += 1
                return ps

            def epilogue(ps, jo, half):
                ot = outs.tile([128, 512], F32, tag="ot")
                nc.scalar.activation(
                    out=ot,
                    in_=ps,
                    func=AF.Identity,
                    bias=yb_sb[:, NCI + jo : NCI + jo + 1],
                    scale=rs_sb[:, jo : jo + 1],
                )
                nc.sync.dma_start(
                    out=out_ext[jo, :, half * 512 : (half + 1) * 512], in_=ot
                )

            # NOTE: emission order IS dataflow order under Tile. The first
            # co-tile's matmuls are emitted before the demod chain so the PE
            # starts as soon as x + the first weight half land, but their
            # epilogues (which read rs_sb) must come after the demod chain.
            ps00 = conv_mms(0, 0)
            ps01 = conv_mms(0, 1)
            for jo in range(NCO):
                for j in range(NCI):
                    nc.tensor.matmul(
                        out=xs2_ps[:, jo : jo + 1],
                        lhsT=wt_slice(j, jo, KK),
                        rhs=ys2_sb[:, j : j + 1],
                        start=(j == 0),
                        stop=(j == NCI - 1),
                    )
            nc.scalar.activation(out=rs_sb, in_=xs2_ps, func=AF.Sqrt, bias=eps_sb)
            nc.vector.reciprocal(out=rs_sb, in_=rs_sb)
            epilogue(ps00, 0, 0)
            epilogue(ps01, 0, 1)
            for jo in range(1, NCO):
                for half in range(2):
                    epilogue(conv_mms(jo, half), jo, half)
            # keep the warm-up matmuls live (cheap PSUM read at the end)
            warm_sink = singles.tile([1, 1], F32)
            nc.vector.tensor_copy(out=warm_sink, in_=warm_ps[0:1, 0:1])
    nc.compile()
    return nc


_NC_CACHE = None


def _get_nc():
    global _NC_CACHE
    if _NC_CACHE is None:
        _NC_CACHE = build_nc()
    return _NC_CACHE


def _prep_inputs(x, y_s, weight, bias):
    # [co, ci, kh, kw] -> [k, ci, co]; append w2 = sum_k wt^2 as slot 9;
    # then tile to [jci, jco, ci_p, 10, co_c] bf16 contiguous.
    wt9 = weight.transpose(2, 3, 1, 0).reshape(KK, CI, CO)
    w2 = (wt9.astype(np.float64) ** 2).sum(axis=0).astype(np.float32)
    full = np.concatenate([wt9, w2[None]], axis=0)  # [10, ci, co]
    wtq = np.ascontiguousarray(
        full.reshape(KK + 1, NCI, 128, NCO, 128).transpose(1, 3, 2, 0, 4)
    ).astype(ml_dtypes.bfloat16)
    in_maps = []
    for b in range(B):
        yb = np.empty((128, 2 * NCI), np.float32)
        yb[:, :NCI] = y_s[b].reshape(NCI, 128).T
        yb[:, NCI:] = bias.reshape(NCO, 128).T
        in_maps.append(
            {
                "x": np.ascontiguousarray(x[b].reshape(NCI, 128, H, W)).astype(
                    ml_dtypes.bfloat16
                ),
                "yb": yb,
                "wt": wtq,
            }
        )
    return in_maps


def _install_trace_support():
    """Dev-only: register the axon NTFF profiling hook + disable the
    remote artifact upload so trace=True works in this container."""
    import sys
    import types

    import concourse.bass_utils as bu

    bu.upload_artifacts = lambda tmpdir: "local://" + str(tmpdir)
    if "antenv.axon_hooks" in sys.modules:
        return
    try:
        from trn_agent_boot.trn_boot import _ntff_profile_via_ctypes

        hook = _ntff_profile_via_ctypes("/opt/axon/libaxon_pjrt.so")
    except Exception:
        return
    mod = types.ModuleType("antenv.axon_hooks")
    mod.get_axon_ntff_profile_hook = lambda: hook
    mod.set_axon_ntff_profile_hook = lambda h: None
    sys.modules["antenv.axon_hooks"] = mod


def run(x, y_s, weight, bias, trace=False, tmpdir=None):
    nc = _get_nc()
    if trace:
        _install_trace_support()
    in_maps = _prep_inputs(x, y_s, weight, bias)
    res = run_bass_kernel_spmd(
        nc, in_maps, core_ids=list(range(B)), trace=trace, tmpdir=tmpdir
    )
    out = np.stack(
        [res.results[b]["out"].reshape(CO, H, W) for b in range(B)]
    ).astype(np.float32)
    return out, res


def kernel(x, y_s, weight, bias):
    out, _ = run(
        np.asarray(x, dtype=np.float32),
        np.asarray(y_s, dtype=np.float32),
        np.asarray(weight, dtype=np.float32),
        np.asarray(bias, dtype=np.float32),
    )
    return out
